# revision 30
# baseline (speedup 1.0000x reference)
"""GATv2 (3 layers, N=50000, E=400000, H=4) on 8 Trainium2 NeuronCores.

Strategy (dst-sharded SPMD):
- Nodes are partitioned across 8 cores (6250 each, padded to 6272 = 49 tiles
  of 128). Each core owns the edges whose dst lands in its slice.
- Per layer: each core projects its local x slice to hs/hd/res (PE matmuls);
  the hs table is AllGather'ed so every core can gather arbitrary src rows.
- Edge phase: per-edge src/dst features arrive via dma_gather (the hs table is
  split at row 32768 into two gather calls because gather indices are int16);
  q = hs_e + hd_e; leaky-relu via ScalarE Prelu(alpha=0.2); attention logits
  via sign-grouped strided reduces (|attn| is folded into the projection
  weights host-side, and within each head the columns are permuted so
  positive-sign columns precede negative ones -> logit = sum(pos) - sum(neg));
  z = exp(logit) (logits are small, no max subtraction needed); messages
  z (x) hs_e; segment-sum via TensorE matmuls with one-hot indicator matrices
  (gathered from a small identity table, so padding slots contribute zero).
- Epilogue: divide by the z-sum (softmax denominator), add residual.
  The |attn| scaling of the output is folded into the next layer's weights;
  the host divides it out of the final layer's output.
"""

import os
import numpy as np
import ml_dtypes

P = 128
H = 4
SPLIT = 32768             # int16 gather split point
CPAD = 256                # gather-table row width (bf16 -> 512B, %256B ok)
bf16 = ml_dtypes.bfloat16


class Cfg:
    def __init__(self, N=50000, E=400000, NC=8, NTILE=49, GT=2):
        self.N, self.E, self.NC, self.NTILE, self.GT = N, E, NC, NTILE, GT
        self.NLOC = N // NC
        self.NPAD = NTILE * 128
        assert self.NLOC <= self.NPAD
        self.NG = NC * self.NPAD
        # (F_in, D_head, C_out) per layer
        self.LAYERS = [(128, 64, 256), (256, 64, 256), (256, 40, 160)]


DEFAULT = Cfg()
_cache = {}
LAST_EXEC_NS = None
LAST_TRACE = None
LAST_RESULTS = None


# ----------------------------------------------------------------------------
# host-side graph prep
# ----------------------------------------------------------------------------

def _lpt_tiles(deg, cfg):
    """Assign NLOC nodes to NTILE tiles (<=128 each), balancing degree sums.
    Returns pos[node] = tile*128 + slot_in_tile."""
    order = np.argsort(-deg, kind="stable")
    loads = np.zeros(cfg.NTILE, np.int64)
    counts = np.zeros(cfg.NTILE, np.int64)
    pos = np.empty(cfg.NLOC, np.int64)
    for v in order:
        avail = counts < 128
        t = np.flatnonzero(avail)[np.argmin(loads[avail])]
        pos[v] = t * 128 + counts[t]
        counts[t] += 1
        loads[t] += deg[v]
    return pos


def _wrap_idx(a):
    """flat int array (len %16==0) -> [128, len/16] int16 wrapped layout."""
    n = a.shape[0]
    w = a.reshape(n // 16, 16).T.astype(np.int16)
    return np.ascontiguousarray(np.tile(w, (8, 1)))


def _prep_graph(src, dst, cfg):
    src = np.asarray(src).astype(np.int64)
    dst = np.asarray(dst).astype(np.int64)
    NC, NLOC, NPAD, NTILE = cfg.NC, cfg.NLOC, cfg.NPAD, cfg.NTILE
    core_d = dst // NLOC
    loc_d = dst % NLOC

    pos_of = np.empty((NC, NLOC), np.int64)
    for c in range(NC):
        deg = np.bincount(loc_d[core_d == c], minlength=NLOC)
        pos_of[c] = _lpt_tiles(deg, cfg)

    src_p = (src // NLOC) * NPAD + pos_of[src // NLOC, src % NLOC]
    dst_pos = pos_of[core_d, loc_d]

    cores = []
    CLs, CHs = [], []
    for c in range(NC):
        m = core_d == c
        sp = src_p[m]
        dp = dst_pos[m]
        tile = dp // 128
        lo = sp < SPLIT
        nL = np.bincount(tile[lo], minlength=NTILE)
        nH = np.bincount(tile[~lo], minlength=NTILE)
        CLs.append(int(np.ceil(nL / 128).max()))
        CHs.append(int(np.ceil(nH / 128).max()))
        cores.append((sp, dp, tile, lo))
    CL = max(max(CLs), 1)
    CH = max(max(CHs), 1)
    NSL, NSH = NTILE * CL * 128, NTILE * CH * 128

    per_core = []
    for c in range(NC):
        sp, dp, tile, lo = cores[c]
        hsL = np.zeros(NSL, np.int64); hdL = np.zeros(NSL, np.int64)
        ohL = np.full(NSL, 128, np.int64)
        hsH = np.zeros(NSH, np.int64); hdH = np.zeros(NSH, np.int64)
        ohH = np.full(NSH, 128, np.int64)
        for t in range(NTILE):
            mt = tile == t
            eL = np.flatnonzero(mt & lo)
            eH = np.flatnonzero(mt & ~lo)
            b = t * CL * 128
            hsL[b:b + eL.size] = sp[eL]
            hdL[b:b + eL.size] = dp[eL]
            ohL[b:b + eL.size] = dp[eL] % 128
            b = t * CH * 128
            hsH[b:b + eH.size] = sp[eH] - SPLIT
            hdH[b:b + eH.size] = dp[eH]
            ohH[b:b + eH.size] = dp[eH] % 128
        # dst_rel as bf16 planes [128, nslots/128]: slot j -> (j%128, j//128);
        # pads hold 128.0 so is_equal against iota 0..127 yields a zero row.
        drelL = np.ascontiguousarray(ohL.reshape(-1, 128).T).astype(bf16)
        drelH = np.ascontiguousarray(ohH.reshape(-1, 128).T).astype(bf16)
        per_core.append({
            "gl_hs": _wrap_idx(hsL), "gh_hs": _wrap_idx(hsH),
            "drel_l": drelL, "drel_h": drelH,
            "drel_fl": np.concatenate([ohL, ohH]).astype(bf16),
        })
    return per_core, pos_of, CL, CH


# ----------------------------------------------------------------------------
# host-side weight prep (fold |attn| + sign permutation into projections)
# ----------------------------------------------------------------------------

def _prep_weights(inp, cfg):
    Ws, phs, rhos, gs = [], [], [], []
    gprev = np.ones(cfg.LAYERS[0][0], np.float64)
    rhoprev = np.arange(cfg.LAYERS[0][0])
    for l, (F, D, C) in enumerate(cfg.LAYERS):
        attn = np.asarray(inp[f"attn{l}"], np.float64)
        aflat = attn.reshape(-1)
        rho = np.empty(C, np.int64)
        ph = []
        for h in range(H):
            colsp = np.flatnonzero(aflat[h * D:(h + 1) * D] > 0) + h * D
            colsn = np.flatnonzero(aflat[h * D:(h + 1) * D] <= 0) + h * D
            ph.append(colsp.size)
            rho[h * D:h * D + colsp.size] = colsp
            rho[h * D + colsp.size:(h + 1) * D] = colsn
        g = np.maximum(np.abs(aflat[rho]), 1e-8)

        ws = np.asarray(inp[f"w_src{l}"], np.float64)
        wd = np.asarray(inp[f"w_dst{l}"], np.float64)
        if f"w_res{l}" in inp:
            wr = np.asarray(inp[f"w_res{l}"], np.float64)
        else:
            wr = np.eye(F, C, dtype=np.float64)

        def dev(w):
            return (w[rhoprev][:, rho] * g[None, :]) / gprev[:, None]

        Ws.append(np.concatenate([dev(ws), dev(wd), dev(wr)], axis=1))
        phs.append(ph)
        rhos.append(rho)
        gs.append(g)
        gprev, rhoprev = g, rho
    return Ws, phs, rhos, gs


# ----------------------------------------------------------------------------
# bass program
# ----------------------------------------------------------------------------

def _build_program(cfg, CL, CH, phs, use_relu=False, stage=3):
    import concourse.mybir as mybir
    import concourse.tile as tile
    from concourse import bacc

    f32 = mybir.dt.float32
    b16 = mybir.dt.bfloat16
    i16 = mybir.dt.int16
    AF = mybir.ActivationFunctionType
    OP = mybir.AluOpType

    NC, NTILE, NPAD, NG, GT = cfg.NC, cfg.NTILE, cfg.NPAD, cfg.NG, cfg.GT
    NSL = NTILE * CL * 128
    NSH = NTILE * CH * 128
    groups = [(g * GT, min(NTILE, (g + 1) * GT))
              for g in range((NTILE + GT - 1) // GT)]
    nhi = max(NG - SPLIT, 1)   # rows in the high half of the hs table

    nc = bacc.Bacc(None, target_bir_lowering=False, debug=False)
    with tile.TileContext(nc) as tc:
        with tc.tile_pool(name="dram", bufs=1, space="DRAM") as dram:
            xT0 = dram.tile([P, NPAD], b16, kind="ExternalInput", name="xT0", uniquify=False)
            wcat = []
            for l, (F, D, C) in enumerate(cfg.LAYERS):
                wcat.append(dram.tile([F, 3 * C], b16, kind="ExternalInput",
                                      name=f"wcat{l}", uniquify=False))
            eye = dram.tile([P, 128], b16, kind="ExternalInput", name="eye", uniquify=False)
            ior = dram.tile([P, 128], b16, kind="ExternalInput", name="ior", uniquify=False)
            gidx = {}
            for nm, sz in [("gl_hs", NSL), ("gh_hs", NSH)]:
                gidx[nm] = dram.tile([P, sz // 16], i16, kind="ExternalInput",
                                     name=nm, uniquify=False)
            drel = {
                "drel_l": dram.tile([P, NSL // 128], b16, kind="ExternalInput",
                                    name="drel_l", uniquify=False),
                "drel_h": dram.tile([P, NSH // 128], b16, kind="ExternalInput",
                                    name="drel_h", uniquify=False),
            }
            drel_fl = dram.tile([NSL + NSH], b16, kind="ExternalInput",
                                name="drel_fl", uniquify=False)
            icol = dram.tile([P, 1], b16, kind="ExternalInput", name="icol", uniquify=False)
            out2 = dram.tile([NPAD, cfg.LAYERS[2][2]], f32, kind="ExternalOutput",
                             name="out2", uniquify=False)

            hs_loc, hs_tbl, hs_hi, x_out = [], [], [], []
            for l in range(3):
                hs_loc.append(dram.tile([NPAD, CPAD], b16, name=f"hs_loc{l}"))
                hs_tbl.append(dram.tile([NG, CPAD], b16, name=f"hs_tbl{l}",
                                        addr_space="Shared"))
                hs_hi.append(dram.tile([nhi, CPAD], b16, name=f"hs_hi{l}"))
                if l < 2:
                    x_out.append(dram.tile([NPAD, 256], b16, name=f"xout{l}"))

            with (
                tc.tile_pool(name="const", bufs=1) as const,
                tc.tile_pool(name="xt", bufs=2) as xtp,
                tc.tile_pool(name="res", bufs=1) as resp,
                tc.tile_pool(name="work", bufs=2) as work,
                tc.tile_pool(name="small", bufs=3) as small,
                tc.tile_pool(name="epi", bufs=4) as epi,
                tc.tile_pool(name="pps", bufs=1, space="PSUM") as pps,
                tc.tile_pool(name="eps", bufs=2, space="PSUM") as eps,
                tc.tile_pool(name="qps", bufs=4, space="PSUM") as qps,
            ):
                eye_sb = const.tile([P, 128], b16, tag="eye")
                nc.sync.dma_start(out=eye_sb[:], in_=eye[:])
                ior_sb = const.tile([P, 1, 128], b16, tag="ior")
                nc.sync.dma_start(out=ior_sb[:, 0, :], in_=ior[:])
                icol_sb = const.tile([P, 1], b16, tag="icol")
                nc.sync.dma_start(out=icol_sb[:], in_=icol[:])
                for l, (F, D, C) in enumerate(cfg.LAYERS):
                    NF = F // 128
                    W = 3 * C
                    xT = xtp.tile([P, NF, NPAD], b16, tag="xT")
                    if l == 0:
                        nc.sync.dma_start(out=xT[:, 0, :], in_=xT0[:])
                    else:
                        xo = x_out[l - 1]
                        nc.sync.dma_start_transpose(out=xT[:, 0, :], in_=xo[:, 0:128])
                        nc.sync.dma_start_transpose(out=xT[:, 1, :], in_=xo[:, 128:256])
                    w_sb = const.tile([P, NF, W], b16, tag="wsb")
                    nc.sync.dma_start(
                        out=w_sb[:], in_=wcat[l][:].rearrange("(f p) w -> p f w", p=P))

                    res_sb = resp.tile([P, NTILE, C], b16 if l < 2 else f32, tag="res")
                    hd_sb = resp.tile([P, NTILE, C], b16, tag="hd")

                    # ---- projections
                    nw = min(W, 512)
                    for t in range(NTILE):
                        pA = pps.tile([P, nw], f32, space="PSUM", tag="pA")
                        if W > 512:
                            pB = pps.tile([P, W - 512], f32, space="PSUM", tag="pB")
                        for fc in range(NF):
                            st, sp_ = (fc == 0), (fc == NF - 1)
                            nc.tensor.matmul(
                                out=pA[:], lhsT=xT[:, fc, t * 128:(t + 1) * 128],
                                rhs=w_sb[:, fc, 0:nw], start=st, stop=sp_)
                        if W > 512:
                            for fc in range(NF):
                                st, sp_ = (fc == 0), (fc == NF - 1)
                                nc.tensor.matmul(
                                    out=pB[:], lhsT=xT[:, fc, t * 128:(t + 1) * 128],
                                    rhs=w_sb[:, fc, 512:W], start=st, stop=sp_)
                        hsrow = epi.tile([P, CPAD], b16, tag="hsrow")
                        nc.scalar.copy(out=hsrow[:, 0:C], in_=pA[:, 0:C])
                        if C < CPAD:
                            nc.vector.memset(hsrow[:, C:CPAD], 0.0)
                        hdsrc = pA[:, C:2 * C]
                        ressrc = pB[:, 0:C] if W > 512 else pA[:, 2 * C:3 * C]
                        nc.vector.tensor_copy(out=hd_sb[:, t, :], in_=hdsrc)
                        nc.vector.tensor_copy(out=res_sb[:, t, :], in_=ressrc)
                        nc.sync.dma_start(
                            out=hs_loc[l][:].rearrange("(t p) c -> p t c", p=P)[:, t, :],
                            in_=hsrow[:])

                    nc.gpsimd.collective_compute(
                        "AllGather", OP.bypass,
                        replica_groups=[list(range(NC))],
                        ins=[hs_loc[l][:]], outs=[hs_tbl[l][:]],
                    )
                    if NG > SPLIT:
                        # dma_gather cannot read from a row-offset slice
                        # (device fault) -> keep a base-aligned copy of the
                        # high half of the table.
                        nc.sync.dma_start(out=hs_hi[l][:],
                                          in_=hs_tbl[l][SPLIT:NG, :])

                    # ---- edge phase
                    if stage == 1:
                        if l == 2:
                            for t in range(NTILE):
                                ot = epi.tile([P, C], f32, tag="osb")
                                nc.vector.tensor_copy(out=ot[:], in_=res_sb[:, t, :])
                                nc.sync.dma_start(
                                    out=out2[:].rearrange("(t p) c -> p t c", p=P)[:, t, :],
                                    in_=ot[:])
                        continue
                    for (t0, t1) in groups:
                        nt = t1 - t0
                        nbL, nbH = nt * CL, nt * CH
                        NB = nbL + nbH
                        qA = work.tile([P, NB, CPAD], b16, tag="qA")
                        qB = work.tile([P, NB, CPAD], b16, tag="qB")
                        oh = work.tile([P, NB, 128], b16, tag="oh")
                        rhs = work.tile([P, NB, 4 + C], b16, tag="rhs")
                        idxs = {}
                        for nm, cnt, off in [
                            ("gl_hs", nbL * 8, t0 * CL * 8), ("gh_hs", nbH * 8, t0 * CH * 8),
                        ]:
                            it = small.tile([P, cnt], i16, tag=nm)
                            nc.sync.dma_start(out=it[:], in_=gidx[nm][:, off:off + cnt])
                            idxs[nm] = it
                        dr = small.tile([P, NB], b16, tag="dr")
                        nc.sync.dma_start(out=dr[:, 0:nbL],
                                          in_=drel["drel_l"][:, t0 * CL:t0 * CL + nbL])
                        nc.sync.dma_start(out=dr[:, nbL:NB],
                                          in_=drel["drel_h"][:, t0 * CH:t0 * CH + nbH])
                        nc.gpsimd.dma_gather(
                            out_ap=qA[:, 0:nbL, :], in_ap=hs_tbl[l][:],
                            idxs_ap=idxs["gl_hs"][:], num_idxs=nbL * 128,
                            num_idxs_reg=nbL * 128, elem_size=CPAD, single_packet=False)
                        nc.gpsimd.dma_gather(
                            out_ap=qA[:, nbL:NB, :],
                            in_ap=(hs_hi[l][:] if NG > SPLIT else hs_tbl[l][:]),
                            idxs_ap=idxs["gh_hs"][:], num_idxs=nbH * 128,
                            num_idxs_reg=nbH * 128, elem_size=CPAD, single_packet=False)
                        # one-hot indicators: S.T[j, i] = (dst_rel[j] == i)
                        nc.vector.tensor_tensor(
                            out=oh[:],
                            in0=dr[:].to_broadcast([P, NB, 128]),
                            in1=ior_sb[:].to_broadcast([P, NB, 128]),
                            op=OP.is_equal)
                        # S[i, j] = (i == dst_rel[j]) via row-replicated drel
                        from concourse.bass import AP as _AP
                        drfl_l = drel_fl[t0 * CL * 128:t0 * CL * 128 + nbL * 128]
                        drfl_h = drel_fl[NSL + t0 * CH * 128:NSL + t0 * CH * 128 + nbH * 128]
                        dre = work.tile([P, NB, 128], b16, tag="dre")
                        nc.sync.dma_start(
                            out=dre[:, 0:nbL, :],
                            in_=_AP(drfl_l.tensor, drfl_l.offset,
                                    [[0, P], [128, nbL], [1, 128]]))
                        nc.sync.dma_start(
                            out=dre[:, nbL:NB, :],
                            in_=_AP(drfl_h.tensor, drfl_h.offset,
                                    [[0, P], [128, nbH], [1, 128]]))
                        smat = work.tile([P, NB, 128], b16, tag="smat")
                        nc.vector.tensor_tensor(
                            out=smat[:],
                            in0=dre[:],
                            in1=icol_sb[:].to_broadcast([P, NB, 128]),
                            op=OP.is_equal)

                        if stage == 20:
                            if l == 2:
                                for tl in range(nt):
                                    ot = epi.tile([P, C], f32, tag="osb")
                                    nc.vector.tensor_copy(
                                        out=ot[:], in_=qA[:, tl * CL, 0:C])
                                    nc.sync.dma_start(
                                        out=out2[:].rearrange("(t p) c -> p t c", p=P)[:, t0 + tl, :],
                                        in_=ot[:])
                            continue
                        for bi in range(NB):
                            tt = t0 + (bi // CL if bi < nbL else (bi - nbL) // CH)
                            qp = qps.tile([P, C], f32, space="PSUM", tag="qps")
                            nc.tensor.matmul(out=qp[:], lhsT=smat[:, bi, :],
                                             rhs=hd_sb[:, tt, :],
                                             start=True, stop=False)
                            nc.tensor.matmul(out=qp[:], lhsT=eye_sb[:],
                                             rhs=qA[:, bi, 0:C],
                                             start=False, stop=True)
                            if use_relu:
                                nc.scalar.activation(out=qB[:, bi, 0:C], in_=qp[:],
                                                     func=AF.Relu)
                            else:
                                nc.scalar.activation(out=qB[:, bi, 0:C], in_=qp[:],
                                                     func=AF.Prelu, alpha=0.2)
                        if stage == 21:
                            if l == 2:
                                for tl in range(nt):
                                    ot = epi.tile([P, C], f32, tag="osb")
                                    nc.vector.tensor_copy(
                                        out=ot[:], in_=qB[:, tl * CL, 0:C])
                                    nc.sync.dma_start(
                                        out=out2[:].rearrange("(t p) c -> p t c", p=P)[:, t0 + tl, :],
                                        in_=ot[:])
                            continue
                        red = small.tile([P, 2, NB, H], f32, tag="red")
                        for h in range(H):
                            p = phs[l][h]
                            if p > 0:
                                nc.vector.tensor_reduce(
                                    out=red[:, 0, :, h], in_=qB[:, :, h * D:h * D + p],
                                    axis=mybir.AxisListType.X, op=OP.add)
                            else:
                                nc.vector.memset(red[:, 0, :, h], 0.0)
                            if p < D:
                                nc.vector.tensor_reduce(
                                    out=red[:, 1, :, h], in_=qB[:, :, h * D + p:(h + 1) * D],
                                    axis=mybir.AxisListType.X, op=OP.add)
                            else:
                                nc.vector.memset(red[:, 1, :, h], 0.0)
                        lg = small.tile([P, NB, H], f32, tag="lg")
                        nc.vector.tensor_tensor(
                            out=lg[:].rearrange("p b h -> p (b h)"),
                            in0=red[:, 0].rearrange("p b h -> p (b h)"),
                            in1=red[:, 1].rearrange("p b h -> p (b h)"),
                            op=OP.subtract)
                        nc.scalar.activation(
                            out=rhs[:, :, 0:4], in_=lg[:], func=AF.Exp)
                        nc.vector.tensor_tensor(
                            out=rhs[:, :, 4:4 + C].rearrange("p b (h d) -> p b h d", h=H),
                            in0=qA[:, :, 0:C].rearrange("p b (h d) -> p b h d", h=H),
                            in1=rhs[:, :, 0:4].to_broadcast([P, NB, H, D]),
                            op=OP.mult)
                        if stage == 2:
                            # drain: write a slice of rhs so work isn't dead
                            if l == 2:
                                for tl in range(nt):
                                    ot = epi.tile([P, C], f32, tag="osb")
                                    nc.vector.tensor_copy(
                                        out=ot[:], in_=rhs[:, tl * CL, 4:4 + C])
                                    nc.sync.dma_start(
                                        out=out2[:].rearrange("(t p) c -> p t c", p=P)[:, t0 + tl, :],
                                        in_=ot[:])
                            continue
                        for tl in range(nt):
                            t = t0 + tl
                            ps = eps.tile([P, 4 + C], f32, space="PSUM", tag="eps")
                            for k in range(CL):
                                nc.tensor.matmul(
                                    out=ps[:], lhsT=oh[:, tl * CL + k, :],
                                    rhs=rhs[:, tl * CL + k, :],
                                    start=(k == 0), stop=False)
                            for k in range(CH):
                                nc.tensor.matmul(
                                    out=ps[:], lhsT=oh[:, nbL + tl * CH + k, :],
                                    rhs=rhs[:, nbL + tl * CH + k, :],
                                    start=False, stop=(k == CH - 1))
                            sden = epi.tile([P, 4], f32, tag="sden")
                            sinv = epi.tile([P, 4], f32, tag="sinv")
                            nc.vector.tensor_scalar(
                                out=sden[:], in0=ps[:, 0:4], scalar1=1e-20,
                                scalar2=None, op0=OP.add)
                            nc.vector.reciprocal(out=sinv[:], in_=sden[:])
                            osb = epi.tile([P, C], b16 if l < 2 else f32, tag="osb")
                            for h in range(H):
                                nc.vector.tensor_scalar(
                                    out=osb[:, h * D:(h + 1) * D],
                                    in0=ps[:, 4 + h * D:4 + (h + 1) * D],
                                    scalar1=sinv[:, h:h + 1], scalar2=None,
                                    op0=OP.mult)
                            nc.vector.tensor_tensor(
                                out=osb[:], in0=osb[:], in1=res_sb[:, t, :], op=OP.add)
                            if l < 2:
                                nc.sync.dma_start(
                                    out=x_out[l][:].rearrange("(t p) c -> p t c", p=P)[:, t, :],
                                    in_=osb[:])
                            else:
                                nc.sync.dma_start(
                                    out=out2[:].rearrange("(t p) c -> p t c", p=P)[:, t, :],
                                    in_=osb[:])
    nc.compile()
    return nc


# ----------------------------------------------------------------------------
# input assembly (shared by HW run and sim)
# ----------------------------------------------------------------------------

def _make_in_maps(node_inputs, inp, cfg, per_core, pos_of, Ws):
    x0 = np.asarray(node_inputs, np.float64)
    eye = np.eye(128, dtype=bf16)
    ior = np.tile(np.arange(128, dtype=np.float64)[None, :], (128, 1)).astype(bf16)
    in_maps = []
    for c in range(cfg.NC):
        xs = x0[c * cfg.NLOC:(c + 1) * cfg.NLOC]
        xp = np.zeros((cfg.NPAD, cfg.LAYERS[0][0]), np.float64)
        xp[pos_of[c]] = xs
        m = dict(per_core[c])
        m["xT0"] = np.ascontiguousarray(xp.T).astype(bf16)
        for l in range(3):
            m[f"wcat{l}"] = Ws[l].astype(bf16)
        m["eye"] = eye
        m["ior"] = ior
        m["icol"] = np.arange(128, dtype=np.float64).reshape(128, 1).astype(bf16)
        in_maps.append(m)
    return in_maps


def _postprocess(outs, cfg, pos_of, rhos, gs):
    C2 = cfg.LAYERS[2][2]
    full = np.empty((cfg.N, C2), np.float64)
    for c in range(cfg.NC):
        o = np.asarray(outs[c], np.float64)
        full[c * cfg.NLOC:(c + 1) * cfg.NLOC] = o[pos_of[c]]
    x3 = np.empty_like(full)
    x3[:, rhos[2]] = full / gs[2][None, :]
    return x3.reshape(cfg.N, H, cfg.LAYERS[2][1]).mean(axis=1).astype(np.float32)


# ----------------------------------------------------------------------------
# entry point
# ----------------------------------------------------------------------------

def kernel(node_inputs, src, dst, **w):
    from concourse.bass_utils import run_bass_kernel_spmd

    cfg = DEFAULT
    per_core, pos_of, CL, CH = _prep_graph(src, dst, cfg)
    Ws, phs, rhos, gs = _prep_weights(w, cfg)

    stage = int(os.environ.get("BASS_GATV2_STAGE", "3"))
    use_relu = bool(os.environ.get("BASS_GATV2_RELU"))
    key = (CL, CH, tuple(tuple(p) for p in phs), stage, use_relu)
    if key not in _cache:
        _cache[key] = _build_program(cfg, CL, CH, phs, use_relu=use_relu,
                                     stage=stage)
    nc = _cache[key]

    in_maps = _make_in_maps(node_inputs, w, cfg, per_core, pos_of, Ws)

    trace = bool(os.environ.get("BASS_GATV2_TRACE"))
    res = run_bass_kernel_spmd(nc, in_maps, core_ids=list(range(cfg.NC)),
                               trace=trace)
    global LAST_EXEC_NS, LAST_TRACE, LAST_RESULTS
    LAST_EXEC_NS = res.exec_time_ns
    LAST_TRACE = res.instructions_and_trace[1] if res.instructions_and_trace else None
    LAST_RESULTS = res

    return _postprocess([res.results[c]["out2"] for c in range(cfg.NC)],
                        cfg, pos_of, rhos, gs)


# revision 33
# speedup vs baseline: 1.0466x; 1.0466x over previous
"""GATv2 (3 layers, N=50000, E=400000, H=4) on 8 Trainium2 NeuronCores.

Strategy (dst-sharded SPMD):
- Nodes are partitioned across 8 cores (6250 each, padded to 6272 = 49 tiles
  of 128). Each core owns the edges whose dst lands in its slice.
- Per layer: each core projects its local x slice to hs/hd/res (PE matmuls);
  the hs table is AllGather'ed so every core can gather arbitrary src rows.
- Edge phase: per-edge src/dst features arrive via dma_gather (the hs table is
  split at row 32768 into two gather calls because gather indices are int16);
  q = hs_e + hd_e; leaky-relu via ScalarE Prelu(alpha=0.2); attention logits
  via sign-grouped strided reduces (|attn| is folded into the projection
  weights host-side, and within each head the columns are permuted so
  positive-sign columns precede negative ones -> logit = sum(pos) - sum(neg));
  z = exp(logit) (logits are small, no max subtraction needed); messages
  z (x) hs_e; segment-sum via TensorE matmuls with one-hot indicator matrices
  (gathered from a small identity table, so padding slots contribute zero).
- Epilogue: divide by the z-sum (softmax denominator), add residual.
  The |attn| scaling of the output is folded into the next layer's weights;
  the host divides it out of the final layer's output.
"""

import os
import numpy as np
import ml_dtypes

P = 128
H = 4
SPLIT = 32768             # int16 gather split point
CPAD = 256                # gather-table row width (bf16 -> 512B, %256B ok)
bf16 = ml_dtypes.bfloat16


class Cfg:
    def __init__(self, N=50000, E=400000, NC=8, NTILE=49, GT=2):
        self.N, self.E, self.NC, self.NTILE, self.GT = N, E, NC, NTILE, GT
        self.NLOC = N // NC
        self.NPAD = NTILE * 128
        assert self.NLOC <= self.NPAD
        self.NG = NC * self.NPAD
        # (F_in, D_head, C_out) per layer
        self.LAYERS = [(128, 64, 256), (256, 64, 256), (256, 40, 160)]


DEFAULT = Cfg()
_cache = {}
LAST_EXEC_NS = None
LAST_TRACE = None
LAST_RESULTS = None


# ----------------------------------------------------------------------------
# host-side graph prep
# ----------------------------------------------------------------------------

def _lpt_tiles(deg, cfg):
    """Assign NLOC nodes to NTILE tiles (<=128 each), balancing degree sums.
    Returns pos[node] = tile*128 + slot_in_tile."""
    order = np.argsort(-deg, kind="stable")
    loads = np.zeros(cfg.NTILE, np.int64)
    counts = np.zeros(cfg.NTILE, np.int64)
    pos = np.empty(cfg.NLOC, np.int64)
    for v in order:
        avail = counts < 128
        t = np.flatnonzero(avail)[np.argmin(loads[avail])]
        pos[v] = t * 128 + counts[t]
        counts[t] += 1
        loads[t] += deg[v]
    return pos


def _wrap_idx(a):
    """flat int array (len %16==0) -> [128, len/16] int16 wrapped layout."""
    n = a.shape[0]
    w = a.reshape(n // 16, 16).T.astype(np.int16)
    return np.ascontiguousarray(np.tile(w, (8, 1)))


def _prep_graph(src, dst, cfg):
    src = np.asarray(src).astype(np.int64)
    dst = np.asarray(dst).astype(np.int64)
    NC, NLOC, NPAD, NTILE = cfg.NC, cfg.NLOC, cfg.NPAD, cfg.NTILE
    core_d = dst // NLOC
    loc_d = dst % NLOC

    pos_of = np.empty((NC, NLOC), np.int64)
    for c in range(NC):
        deg = np.bincount(loc_d[core_d == c], minlength=NLOC)
        pos_of[c] = _lpt_tiles(deg, cfg)

    src_p = (src // NLOC) * NPAD + pos_of[src // NLOC, src % NLOC]
    dst_pos = pos_of[core_d, loc_d]

    cores = []
    CLs, CHs = [], []
    for c in range(NC):
        m = core_d == c
        sp = src_p[m]
        dp = dst_pos[m]
        tile = dp // 128
        lo = sp < SPLIT
        nL = np.bincount(tile[lo], minlength=NTILE)
        nH = np.bincount(tile[~lo], minlength=NTILE)
        CLs.append(int(np.ceil(nL / 128).max()))
        CHs.append(int(np.ceil(nH / 128).max()))
        cores.append((sp, dp, tile, lo))
    CL = max(max(CLs), 1)
    CH = max(max(CHs), 1)
    NSL, NSH = NTILE * CL * 128, NTILE * CH * 128

    per_core = []
    for c in range(NC):
        sp, dp, tile, lo = cores[c]
        hsL = np.zeros(NSL, np.int64); hdL = np.zeros(NSL, np.int64)
        ohL = np.full(NSL, 128, np.int64)
        hsH = np.zeros(NSH, np.int64); hdH = np.zeros(NSH, np.int64)
        ohH = np.full(NSH, 128, np.int64)
        for t in range(NTILE):
            mt = tile == t
            eL = np.flatnonzero(mt & lo)
            eH = np.flatnonzero(mt & ~lo)
            b = t * CL * 128
            hsL[b:b + eL.size] = sp[eL]
            hdL[b:b + eL.size] = dp[eL]
            ohL[b:b + eL.size] = dp[eL] % 128
            b = t * CH * 128
            hsH[b:b + eH.size] = sp[eH] - SPLIT
            hdH[b:b + eH.size] = dp[eH]
            ohH[b:b + eH.size] = dp[eH] % 128
        # dst_rel as bf16 planes [128, nslots/128]: slot j -> (j%128, j//128);
        # pads hold 128.0 so is_equal against iota 0..127 yields a zero row.
        drelL = np.ascontiguousarray(ohL.reshape(-1, 128).T).astype(bf16)
        drelH = np.ascontiguousarray(ohH.reshape(-1, 128).T).astype(bf16)
        per_core.append({
            "gl_hs": _wrap_idx(hsL), "gh_hs": _wrap_idx(hsH),
            "drel_l": drelL, "drel_h": drelH,
            "drel_fl": np.concatenate([ohL, ohH]).astype(bf16),
        })
    return per_core, pos_of, CL, CH


# ----------------------------------------------------------------------------
# host-side weight prep (fold |attn| + sign permutation into projections)
# ----------------------------------------------------------------------------

def _prep_weights(inp, cfg):
    Ws, phs, rhos, gs = [], [], [], []
    gprev = np.ones(cfg.LAYERS[0][0], np.float64)
    rhoprev = np.arange(cfg.LAYERS[0][0])
    for l, (F, D, C) in enumerate(cfg.LAYERS):
        attn = np.asarray(inp[f"attn{l}"], np.float64)
        aflat = attn.reshape(-1)
        rho = np.empty(C, np.int64)
        ph = []
        for h in range(H):
            colsp = np.flatnonzero(aflat[h * D:(h + 1) * D] > 0) + h * D
            colsn = np.flatnonzero(aflat[h * D:(h + 1) * D] <= 0) + h * D
            ph.append(colsp.size)
            rho[h * D:h * D + colsp.size] = colsp
            rho[h * D + colsp.size:(h + 1) * D] = colsn
        g = np.maximum(np.abs(aflat[rho]), 1e-8)

        ws = np.asarray(inp[f"w_src{l}"], np.float64)
        wd = np.asarray(inp[f"w_dst{l}"], np.float64)
        if f"w_res{l}" in inp:
            wr = np.asarray(inp[f"w_res{l}"], np.float64)
        else:
            wr = np.eye(F, C, dtype=np.float64)

        def dev(w):
            return (w[rhoprev][:, rho] * g[None, :]) / gprev[:, None]

        Ws.append(np.concatenate([dev(ws), dev(wd), dev(wr)], axis=1))
        phs.append(ph)
        rhos.append(rho)
        gs.append(g)
        gprev, rhoprev = g, rho
    return Ws, phs, rhos, gs


# ----------------------------------------------------------------------------
# bass program
# ----------------------------------------------------------------------------

def _build_program(cfg, CL, CH, phs, use_relu=False, stage=3):
    import concourse.mybir as mybir
    import concourse.tile as tile
    from concourse import bacc

    f32 = mybir.dt.float32
    b16 = mybir.dt.bfloat16
    i16 = mybir.dt.int16
    AF = mybir.ActivationFunctionType
    OP = mybir.AluOpType

    NC, NTILE, NPAD, NG, GT = cfg.NC, cfg.NTILE, cfg.NPAD, cfg.NG, cfg.GT
    NSL = NTILE * CL * 128
    NSH = NTILE * CH * 128
    groups = [(g * GT, min(NTILE, (g + 1) * GT))
              for g in range((NTILE + GT - 1) // GT)]
    nhi = max(NG - SPLIT, 1)   # rows in the high half of the hs table

    nc = bacc.Bacc(None, target_bir_lowering=False, debug=False)
    with tile.TileContext(nc) as tc:
        with tc.tile_pool(name="dram", bufs=1, space="DRAM") as dram:
            xT0 = dram.tile([P, NPAD], b16, kind="ExternalInput", name="xT0", uniquify=False)
            wcat = []
            for l, (F, D, C) in enumerate(cfg.LAYERS):
                wcat.append(dram.tile([F, 3 * C], b16, kind="ExternalInput",
                                      name=f"wcat{l}", uniquify=False))
            eye = dram.tile([P, 128], b16, kind="ExternalInput", name="eye", uniquify=False)
            ior = dram.tile([P, 128], b16, kind="ExternalInput", name="ior", uniquify=False)
            gidx = {}
            for nm, sz in [("gl_hs", NSL), ("gh_hs", NSH)]:
                gidx[nm] = dram.tile([P, sz // 16], i16, kind="ExternalInput",
                                     name=nm, uniquify=False)
            drel = {
                "drel_l": dram.tile([P, NSL // 128], b16, kind="ExternalInput",
                                    name="drel_l", uniquify=False),
                "drel_h": dram.tile([P, NSH // 128], b16, kind="ExternalInput",
                                    name="drel_h", uniquify=False),
            }
            drel_fl = dram.tile([NSL + NSH], b16, kind="ExternalInput",
                                name="drel_fl", uniquify=False)
            icol = dram.tile([P, 1], b16, kind="ExternalInput", name="icol", uniquify=False)
            out2 = dram.tile([NPAD, cfg.LAYERS[2][2]], f32, kind="ExternalOutput",
                             name="out2", uniquify=False)

            hs_loc, hs_tbl, hs_hi, x_out = [], [], [], []
            for l in range(3):
                hs_loc.append(dram.tile([NPAD, CPAD], b16, name=f"hs_loc{l}"))
                hs_tbl.append(dram.tile([NG, CPAD], b16, name=f"hs_tbl{l}",
                                        addr_space="Shared"))
                hs_hi.append(dram.tile([nhi, CPAD], b16, name=f"hs_hi{l}"))
                if l < 2:
                    x_out.append(dram.tile([NPAD, 256], b16, name=f"xout{l}"))

            with (
                tc.tile_pool(name="const", bufs=1) as const,
                tc.tile_pool(name="xt", bufs=1) as xtp,
                tc.tile_pool(name="res", bufs=1) as resp,
                tc.tile_pool(name="work", bufs=2) as work,
                tc.tile_pool(name="small", bufs=3) as small,
                tc.tile_pool(name="epi", bufs=4) as epi,
                tc.tile_pool(name="pps", bufs=1, space="PSUM") as pps,
                tc.tile_pool(name="eps", bufs=2, space="PSUM") as eps,
                tc.tile_pool(name="qps", bufs=4, space="PSUM") as qps,
            ):
                eye_sb = const.tile([P, 128], b16, tag="eye")
                nc.sync.dma_start(out=eye_sb[:], in_=eye[:])
                ior_sb = const.tile([P, 1, 128], b16, tag="ior")
                nc.sync.dma_start(out=ior_sb[:, 0, :], in_=ior[:])
                icol_sb = const.tile([P, 1], b16, tag="icol")
                nc.sync.dma_start(out=icol_sb[:], in_=icol[:])
                xTs = {}
                for l, (F, D, C) in enumerate(cfg.LAYERS):
                    NF = F // 128
                    W = 3 * C
                    if l == 0:
                        xT = xtp.tile([P, NF, NPAD], b16, tag="xT0t")
                        nc.sync.dma_start(out=xT[:, 0, :], in_=xT0[:])
                    else:
                        xT = xTs[l]   # filled by layer l-1's edge loop
                    w_sb = const.tile([P, NF, W], b16, tag="wsb")
                    nc.sync.dma_start(
                        out=w_sb[:], in_=wcat[l][:].rearrange("(f p) w -> p f w", p=P))

                    res_sb = resp.tile([P, NTILE, C], b16, tag="res")
                    hd_sb = resp.tile([P, NTILE, C], b16, tag="hd")

                    # ---- projections
                    nw = min(W, 512)
                    for t in range(NTILE):
                        pA = pps.tile([P, nw], f32, space="PSUM", tag="pA")
                        if W > 512:
                            pB = pps.tile([P, W - 512], f32, space="PSUM", tag="pB")
                        for fc in range(NF):
                            st, sp_ = (fc == 0), (fc == NF - 1)
                            nc.tensor.matmul(
                                out=pA[:], lhsT=xT[:, fc, t * 128:(t + 1) * 128],
                                rhs=w_sb[:, fc, 0:nw], start=st, stop=sp_)
                        if W > 512:
                            for fc in range(NF):
                                st, sp_ = (fc == 0), (fc == NF - 1)
                                nc.tensor.matmul(
                                    out=pB[:], lhsT=xT[:, fc, t * 128:(t + 1) * 128],
                                    rhs=w_sb[:, fc, 512:W], start=st, stop=sp_)
                        hsrow = epi.tile([P, CPAD], b16, tag="hsrow")
                        nc.scalar.copy(out=hsrow[:, 0:C], in_=pA[:, 0:C])
                        if C < CPAD:
                            nc.vector.memset(hsrow[:, C:CPAD], 0.0)
                        hdsrc = pA[:, C:2 * C]
                        ressrc = pB[:, 0:C] if W > 512 else pA[:, 2 * C:3 * C]
                        nc.vector.tensor_copy(out=hd_sb[:, t, :], in_=hdsrc)
                        nc.vector.tensor_copy(out=res_sb[:, t, :], in_=ressrc)
                        nc.sync.dma_start(
                            out=hs_loc[l][:].rearrange("(t p) c -> p t c", p=P)[:, t, :],
                            in_=hsrow[:])

                    nc.gpsimd.collective_compute(
                        "AllGather", OP.bypass,
                        replica_groups=[list(range(NC))],
                        ins=[hs_loc[l][:]], outs=[hs_tbl[l][:]],
                    )
                    if NG > SPLIT:
                        # dma_gather cannot read from a row-offset slice
                        # (device fault) -> keep a base-aligned copy of the
                        # high half of the table.
                        nc.sync.dma_start(out=hs_hi[l][:],
                                          in_=hs_tbl[l][SPLIT:NG, :])

                    # ---- edge phase
                    if l < 2:
                        xTs[l + 1] = xtp.tile([P, 2, NPAD], b16, tag="xTn", name=f"xTn{l + 1}")
                    if stage == 1:
                        if l == 2:
                            for t in range(NTILE):
                                ot = epi.tile([P, C], f32, tag="osb")
                                nc.vector.tensor_copy(out=ot[:], in_=res_sb[:, t, :])
                                nc.sync.dma_start(
                                    out=out2[:].rearrange("(t p) c -> p t c", p=P)[:, t, :],
                                    in_=ot[:])
                        continue
                    for (t0, t1) in groups:
                        nt = t1 - t0
                        nbL, nbH = nt * CL, nt * CH
                        NB = nbL + nbH
                        qA = work.tile([P, NB, CPAD], b16, tag="qA")
                        qB = work.tile([P, NB, CPAD], b16, tag="qB")
                        oh = work.tile([P, NB, 128], b16, tag="oh")
                        rhs = work.tile([P, NB, 4 + C], b16, tag="rhs")
                        idxs = {}
                        for nm, cnt, off in [
                            ("gl_hs", nbL * 8, t0 * CL * 8), ("gh_hs", nbH * 8, t0 * CH * 8),
                        ]:
                            it = small.tile([P, cnt], i16, tag=nm)
                            nc.sync.dma_start(out=it[:], in_=gidx[nm][:, off:off + cnt])
                            idxs[nm] = it
                        dr = small.tile([P, NB], b16, tag="dr")
                        nc.sync.dma_start(out=dr[:, 0:nbL],
                                          in_=drel["drel_l"][:, t0 * CL:t0 * CL + nbL])
                        nc.sync.dma_start(out=dr[:, nbL:NB],
                                          in_=drel["drel_h"][:, t0 * CH:t0 * CH + nbH])
                        nc.gpsimd.dma_gather(
                            out_ap=qA[:, 0:nbL, :], in_ap=hs_tbl[l][:],
                            idxs_ap=idxs["gl_hs"][:], num_idxs=nbL * 128,
                            num_idxs_reg=nbL * 128, elem_size=CPAD, single_packet=False)
                        nc.gpsimd.dma_gather(
                            out_ap=qA[:, nbL:NB, :],
                            in_ap=(hs_hi[l][:] if NG > SPLIT else hs_tbl[l][:]),
                            idxs_ap=idxs["gh_hs"][:], num_idxs=nbH * 128,
                            num_idxs_reg=nbH * 128, elem_size=CPAD, single_packet=False)
                        # one-hot indicators: S.T[j, i] = (dst_rel[j] == i)
                        nc.vector.tensor_tensor(
                            out=oh[:],
                            in0=dr[:].to_broadcast([P, NB, 128]),
                            in1=ior_sb[:].to_broadcast([P, NB, 128]),
                            op=OP.is_equal)
                        # S[i, j] = (i == dst_rel[j]) via row-replicated drel
                        from concourse.bass import AP as _AP
                        drfl_l = drel_fl[t0 * CL * 128:t0 * CL * 128 + nbL * 128]
                        drfl_h = drel_fl[NSL + t0 * CH * 128:NSL + t0 * CH * 128 + nbH * 128]
                        dre = work.tile([P, NB, 128], b16, tag="dre")
                        nc.sync.dma_start(
                            out=dre[:, 0:nbL, :],
                            in_=_AP(drfl_l.tensor, drfl_l.offset,
                                    [[0, P], [128, nbL], [1, 128]]))
                        nc.sync.dma_start(
                            out=dre[:, nbL:NB, :],
                            in_=_AP(drfl_h.tensor, drfl_h.offset,
                                    [[0, P], [128, nbH], [1, 128]]))
                        smat = dre
                        nc.vector.tensor_tensor(
                            out=smat[:],
                            in0=dre[:],
                            in1=icol_sb[:].to_broadcast([P, NB, 128]),
                            op=OP.is_equal)

                        if stage == 20:
                            if l == 2:
                                for tl in range(nt):
                                    ot = epi.tile([P, C], f32, tag="osb")
                                    nc.vector.tensor_copy(
                                        out=ot[:], in_=qA[:, tl * CL, 0:C])
                                    nc.sync.dma_start(
                                        out=out2[:].rearrange("(t p) c -> p t c", p=P)[:, t0 + tl, :],
                                        in_=ot[:])
                            continue
                        for bi in range(NB):
                            tt = t0 + (bi // CL if bi < nbL else (bi - nbL) // CH)
                            qp = qps.tile([P, C], f32, space="PSUM", tag="qps")
                            nc.tensor.matmul(out=qp[:], lhsT=smat[:, bi, :],
                                             rhs=hd_sb[:, tt, :],
                                             start=True, stop=False)
                            nc.tensor.matmul(out=qp[:], lhsT=eye_sb[:],
                                             rhs=qA[:, bi, 0:C],
                                             start=False, stop=True)
                            if use_relu:
                                nc.scalar.activation(out=qB[:, bi, 0:C], in_=qp[:],
                                                     func=AF.Relu)
                            else:
                                nc.scalar.activation(out=qB[:, bi, 0:C], in_=qp[:],
                                                     func=AF.Prelu, alpha=0.2)
                        if stage == 21:
                            if l == 2:
                                for tl in range(nt):
                                    ot = epi.tile([P, C], f32, tag="osb")
                                    nc.vector.tensor_copy(
                                        out=ot[:], in_=qB[:, tl * CL, 0:C])
                                    nc.sync.dma_start(
                                        out=out2[:].rearrange("(t p) c -> p t c", p=P)[:, t0 + tl, :],
                                        in_=ot[:])
                            continue
                        red = small.tile([P, 2, NB, H], f32, tag="red")
                        for h in range(H):
                            p = phs[l][h]
                            if p > 0:
                                nc.vector.tensor_reduce(
                                    out=red[:, 0, :, h], in_=qB[:, :, h * D:h * D + p],
                                    axis=mybir.AxisListType.X, op=OP.add)
                            else:
                                nc.vector.memset(red[:, 0, :, h], 0.0)
                            if p < D:
                                nc.vector.tensor_reduce(
                                    out=red[:, 1, :, h], in_=qB[:, :, h * D + p:(h + 1) * D],
                                    axis=mybir.AxisListType.X, op=OP.add)
                            else:
                                nc.vector.memset(red[:, 1, :, h], 0.0)
                        lg = small.tile([P, NB, H], f32, tag="lg")
                        nc.vector.tensor_tensor(
                            out=lg[:].rearrange("p b h -> p (b h)"),
                            in0=red[:, 0].rearrange("p b h -> p (b h)"),
                            in1=red[:, 1].rearrange("p b h -> p (b h)"),
                            op=OP.subtract)
                        nc.scalar.activation(
                            out=rhs[:, :, 0:4], in_=lg[:], func=AF.Exp)
                        nc.vector.tensor_tensor(
                            out=rhs[:, :, 4:4 + C].rearrange("p b (h d) -> p b h d", h=H),
                            in0=qA[:, :, 0:C].rearrange("p b (h d) -> p b h d", h=H),
                            in1=rhs[:, :, 0:4].to_broadcast([P, NB, H, D]),
                            op=OP.mult)
                        if stage == 2:
                            # drain: write a slice of rhs so work isn't dead
                            if l == 2:
                                for tl in range(nt):
                                    ot = epi.tile([P, C], f32, tag="osb")
                                    nc.vector.tensor_copy(
                                        out=ot[:], in_=rhs[:, tl * CL, 4:4 + C])
                                    nc.sync.dma_start(
                                        out=out2[:].rearrange("(t p) c -> p t c", p=P)[:, t0 + tl, :],
                                        in_=ot[:])
                            continue
                        for tl in range(nt):
                            t = t0 + tl
                            ps = eps.tile([P, 4 + C], f32, space="PSUM", tag="eps")
                            for k in range(CL):
                                nc.tensor.matmul(
                                    out=ps[:], lhsT=oh[:, tl * CL + k, :],
                                    rhs=rhs[:, tl * CL + k, :],
                                    start=(k == 0), stop=False)
                            for k in range(CH):
                                nc.tensor.matmul(
                                    out=ps[:], lhsT=oh[:, nbL + tl * CH + k, :],
                                    rhs=rhs[:, nbL + tl * CH + k, :],
                                    start=False, stop=(k == CH - 1))
                            sden = epi.tile([P, 4], f32, tag="sden")
                            sinv = epi.tile([P, 4], f32, tag="sinv")
                            nc.vector.tensor_scalar(
                                out=sden[:], in0=ps[:, 0:4], scalar1=1e-20,
                                scalar2=None, op0=OP.add)
                            nc.vector.reciprocal(out=sinv[:], in_=sden[:])
                            osb = epi.tile([P, C], b16 if l < 2 else f32, tag="osb")
                            for h in range(H):
                                nc.vector.tensor_scalar(
                                    out=osb[:, h * D:(h + 1) * D],
                                    in0=ps[:, 4 + h * D:4 + (h + 1) * D],
                                    scalar1=sinv[:, h:h + 1], scalar2=None,
                                    op0=OP.mult)
                            nc.vector.tensor_tensor(
                                out=osb[:], in0=osb[:], in1=res_sb[:, t, :], op=OP.add)
                            if l < 2:
                                nc.sync.dma_start(
                                    out=x_out[l][:].rearrange("(t p) c -> p t c", p=P)[:, t, :],
                                    in_=osb[:])
                            else:
                                nc.sync.dma_start(
                                    out=out2[:].rearrange("(t p) c -> p t c", p=P)[:, t, :],
                                    in_=osb[:])
                        if l < 2:
                            rows = slice(t0 * 128, t1 * 128)
                            nxT = xTs[l + 1]
                            nc.sync.dma_start_transpose(
                                out=nxT[:, 0, t0 * 128:t1 * 128],
                                in_=x_out[l][rows, 0:128])
                            nc.sync.dma_start_transpose(
                                out=nxT[:, 1, t0 * 128:t1 * 128],
                                in_=x_out[l][rows, 128:256])
    nc.compile()
    return nc


# ----------------------------------------------------------------------------
# input assembly (shared by HW run and sim)
# ----------------------------------------------------------------------------

def _make_in_maps(node_inputs, inp, cfg, per_core, pos_of, Ws):
    x0 = np.asarray(node_inputs, np.float64)
    eye = np.eye(128, dtype=bf16)
    ior = np.tile(np.arange(128, dtype=np.float64)[None, :], (128, 1)).astype(bf16)
    in_maps = []
    for c in range(cfg.NC):
        xs = x0[c * cfg.NLOC:(c + 1) * cfg.NLOC]
        xp = np.zeros((cfg.NPAD, cfg.LAYERS[0][0]), np.float64)
        xp[pos_of[c]] = xs
        m = dict(per_core[c])
        m["xT0"] = np.ascontiguousarray(xp.T).astype(bf16)
        for l in range(3):
            m[f"wcat{l}"] = Ws[l].astype(bf16)
        m["eye"] = eye
        m["ior"] = ior
        m["icol"] = np.arange(128, dtype=np.float64).reshape(128, 1).astype(bf16)
        in_maps.append(m)
    return in_maps


def _postprocess(outs, cfg, pos_of, rhos, gs):
    C2 = cfg.LAYERS[2][2]
    full = np.empty((cfg.N, C2), np.float64)
    for c in range(cfg.NC):
        o = np.asarray(outs[c], np.float64)
        full[c * cfg.NLOC:(c + 1) * cfg.NLOC] = o[pos_of[c]]
    x3 = np.empty_like(full)
    x3[:, rhos[2]] = full / gs[2][None, :]
    return x3.reshape(cfg.N, H, cfg.LAYERS[2][1]).mean(axis=1).astype(np.float32)


# ----------------------------------------------------------------------------
# entry point
# ----------------------------------------------------------------------------

def kernel(node_inputs, src, dst, **w):
    from concourse.bass_utils import run_bass_kernel_spmd

    cfg = DEFAULT
    per_core, pos_of, CL, CH = _prep_graph(src, dst, cfg)
    Ws, phs, rhos, gs = _prep_weights(w, cfg)

    stage = int(os.environ.get("BASS_GATV2_STAGE", "3"))
    use_relu = bool(os.environ.get("BASS_GATV2_RELU"))
    key = (CL, CH, tuple(tuple(p) for p in phs), stage, use_relu)
    if key not in _cache:
        _cache[key] = _build_program(cfg, CL, CH, phs, use_relu=use_relu,
                                     stage=stage)
    nc = _cache[key]

    in_maps = _make_in_maps(node_inputs, w, cfg, per_core, pos_of, Ws)

    trace = bool(os.environ.get("BASS_GATV2_TRACE"))
    res = run_bass_kernel_spmd(nc, in_maps, core_ids=list(range(cfg.NC)),
                               trace=trace)
    global LAST_EXEC_NS, LAST_TRACE, LAST_RESULTS
    LAST_EXEC_NS = res.exec_time_ns
    LAST_TRACE = res.instructions_and_trace[1] if res.instructions_and_trace else None
    LAST_RESULTS = res

    return _postprocess([res.results[c]["out2"] for c in range(cfg.NC)],
                        cfg, pos_of, rhos, gs)
